# revision 17
# baseline (speedup 1.0000x reference)
"""Trainium2 Bass kernel for the sparse-attention scoring module.

Math: the reference computes
    s     = concat([h, enc]) @ W_attn.T + b_attn        # [B, T, A]
    score = s @ v                                        # [B, T]
    score = score / weight ; masked -> -1e10 ; softmax over T

Since the A dimension is immediately contracted with v, the big matmul
collapses exactly:  score = concat @ (W_attn.T @ v) + b_attn @ v.
With w = W_attn.T @ v split into w1 (decoder half) and w2 (encoder half):
    score[b, t] = enc[t, b, :] . w2  +  (av[b] . w1 + b.v)
The only large tensor is encoder_outputs (268 MB fp32), so the kernel is
DMA-bound: each of the 8 cores streams its 8-batch shard (33.5 MB) through
SBUF in 512 KB transfers (alternating between the sync and scalar HWDGE DMA
rings, which is what saturates HBM) and does a fused multiply+reduce (STT
with accum) on the vector engine, then a small softmax tail. Scalar prep
(W_attn.T @ v, distance weights, mask penalties) happens on the host and
ships as tiny constant tensors.

Per-core data layout: the shard is re-ordered host-side to b-major rows
[8*1024, 1024] (row i = b*1024 + t). Row-tile j maps partition p to row
i = j*128 + p, i.e. b = j//8, t = (j%8)*128 + p. Scores accumulate into a
[128, 64] tile whose transpose [64, 128] is exactly the [8, 1024] output
row-major, so the final PE transpose + scale writes the output directly.
The -1e10 mask value is folded into the additive init constant as
-1e10 * weight[t], which the 1/weight scale restores to -1e10; exp then
underflows those lanes to exactly 0.
"""

import numpy as np

N_CORES = 8
B, T, E2, D, A = 64, 1024, 1024, 1024, 1024
B_LOC = B // N_CORES          # 8 batch rows per core
ROWS = B_LOC * T              # 8192 rows per core
NT = ROWS // 128              # 64 row-tiles of 128 rows
CHUNK = 1                     # row-tiles per DMA (512 KB transfers)
NEG_INF = -1.0e10

_CACHE = {}


def _build_nc():
    import concourse.bass as bass
    import concourse.tile as tile
    from concourse import bacc, mybir
    from contextlib import ExitStack

    f32 = mybir.dt.float32
    nc = bacc.Bacc("TRN2", target_bir_lowering=False, debug=False,
                   num_devices=N_CORES)

    enc = nc.dram_tensor("enc", [ROWS, E2], f32, kind="ExternalInput").ap()
    w2rep = nc.dram_tensor("w2rep", [128, E2], f32, kind="ExternalInput").ap()
    init = nc.dram_tensor("init", [128, NT], f32, kind="ExternalInput").ap()
    scl = nc.dram_tensor("scl", [128, NT], f32, kind="ExternalInput").ap()
    ones = nc.dram_tensor("ones", [128, 1], f32, kind="ExternalInput").ap()
    sel = nc.dram_tensor("sel", [B_LOC, NT], f32, kind="ExternalInput").ap()
    ident = nc.dram_tensor("ident", [128, 128], f32, kind="ExternalInput").ap()
    out = nc.dram_tensor("out", [NT, 128], f32, kind="ExternalOutput").ap()

    with tile.TileContext(nc) as tc, ExitStack() as ctx:
        const = ctx.enter_context(tc.tile_pool(name="const", bufs=1))
        encp = ctx.enter_context(tc.tile_pool(name="encp", bufs=10))
        prodp = ctx.enter_context(tc.tile_pool(name="prodp", bufs=2))
        small = ctx.enter_context(tc.tile_pool(name="small", bufs=1))
        psump = ctx.enter_context(tc.tile_pool(name="psump", bufs=1, space="PSUM"))

        # w2 replicated across partitions; issued on the sync HWDGE ring
        # while the first enc chunk streams on the scalar ring (the two
        # rings transfer concurrently). Remaining constants ride the SWDGE
        # (gpsimd) ring, off the enc stream.
        w2t = const.tile([128, E2], f32)
        nc.sync.dma_start(w2t[:], w2rep)
        sc = const.tile([128, NT], f32)
        nc.gpsimd.dma_start(sc[:], scl)
        ic = const.tile([128, NT], f32)
        nc.gpsimd.dma_start(ic[:], init)
        on = const.tile([128, 1], f32)
        nc.gpsimd.dma_start(on[:], ones)
        se = const.tile([B_LOC, NT], f32)
        nc.gpsimd.dma_start(se[:], sel)
        idt = const.tile([128, 128], f32)
        nc.gpsimd.dma_start(idt[:], ident)

        # enc chunk list, alternating between the two HWDGE rings.
        chunks = []
        j0 = 0
        while j0 < NT:
            chunks.append((j0, min(CHUNK, NT - j0)))
            j0 += CHUNK
        scores = small.tile([128, NT], f32)
        for ci, (jstart, clen) in enumerate(chunks):
            et = encp.tile([128, CHUNK * E2], f32, tag="enct")
            src = bass.AP(enc.tensor, jstart * 128 * E2,
                          [[E2, 128], [128 * E2, clen], [1, E2]])
            eng = nc.scalar if ci % 2 == 0 else nc.sync
            eng.dma_start(et[:, :clen * E2].rearrange("p (c e) -> p c e", c=clen),
                          src)
            for h in range(clen):
                j = jstart + h
                pr = prodp.tile([128, E2], f32)
                # pr = (et_h * winv_col) * w2 ;  scores[:, j] = sum_e pr
                # (winv[t] is constant per partition within a score column,
                #  so the /weight scale rides the STT's per-partition scalar)
                nc.vector.scalar_tensor_tensor(
                    out=pr[:], in0=et[:, h * E2:(h + 1) * E2],
                    scalar=sc[:, j:j + 1], in1=w2t[:],
                    op0=mybir.AluOpType.mult, op1=mybir.AluOpType.mult,
                    accum_out=scores[:, j:j + 1],
                )

        # softmax tail: score' = scores + init*winv (host-folded); e = exp
        s3 = small.tile([128, NT], f32)
        nc.vector.tensor_add(s3[:], scores[:], ic[:])
        ex = small.tile([128, NT], f32)
        nc.scalar.activation(ex[:], s3[:], mybir.ActivationFunctionType.Exp)
        part = small.tile([128, B_LOC], f32)
        # one 3D-AP reduce: [128, (b thi)] -> sum over thi -> [128, b]
        nc.vector.reduce_sum(part[:], ex[:].rearrange("p (b t) -> p b t", b=B_LOC),
                             axis=mybir.AxisListType.X)
        ptot = psump.tile([B_LOC, 1], f32)
        nc.tensor.matmul(ptot[:], part[:], on[:], start=True, stop=True)
        rtot = small.tile([B_LOC, 1], f32)
        nc.vector.reciprocal(rtot[:], ptot[:])
        p64 = psump.tile([NT, 1], f32)
        nc.tensor.matmul(p64[:], se[:], rtot[:], start=True, stop=True)
        r64 = small.tile([NT, 1], f32)
        nc.scalar.copy(r64[:], p64[:])
        peT = psump.tile([NT, 128], f32)
        nc.tensor.transpose(peT[:], ex[:], idt[:])
        attn = small.tile([NT, 128], f32)
        nc.vector.tensor_scalar_mul(attn[:], peT[:], r64[:])
        nc.sync.dma_start(out, attn[:])

    nc.compile()
    return nc


def _get_nc():
    if "nc" not in _CACHE:
        _CACHE["nc"] = _build_nc()
    return _CACHE["nc"]


def _distance_weight(time_step: int, max_len: int) -> np.ndarray:
    left = np.arange(time_step, 0, -1) + 2
    right = np.arange(max_len - time_step) + 2
    return np.log2(np.concatenate([left, right]).astype(np.float32))


def kernel(attention_vector, encoder_outputs, W_attn, b_attn, v, mask,
           time_step, max_len) -> np.ndarray:
    from concourse.bass_utils import run_bass_kernel_spmd

    av = np.ascontiguousarray(np.asarray(attention_vector, dtype=np.float32))
    enc = np.asarray(encoder_outputs, dtype=np.float32)
    W = np.asarray(W_attn, dtype=np.float32)
    bb = np.asarray(b_attn, dtype=np.float32)
    vv = np.asarray(v, dtype=np.float32)
    mk = np.asarray(mask)
    ts = int(time_step)
    ml = int(max_len)
    assert av.shape == (B, D) and enc.shape == (T, B, E2)
    assert W.shape == (A, D + E2) and mk.shape == (B, T) and ml == T

    # Host-side scalar prep (tiny): collapse W/v/b, distance weights, mask.
    w = W.T @ vv                                   # [D+E2]
    w1, w2 = w[:D], np.ascontiguousarray(w[D:])
    w2t_host = np.ascontiguousarray(np.broadcast_to(w2, (128, E2)))
    bv = np.float32(bb @ vv)
    c1 = (av @ w1 + bv).astype(np.float32)         # [B]
    weight = _distance_weight(ts, ml)              # [T]
    winv = (np.float32(1.0) / weight).astype(np.float32)

    # scl[p, j] = 1/weight[t],  t = (j%8)*128 + p  (same for every b)
    winv2 = winv.reshape(B_LOC, 128).T             # [128, 8] col = thi
    scl = np.ascontiguousarray(np.tile(winv2, (1, NT // B_LOC)))
    ones = np.ones((128, 1), dtype=np.float32)
    sel = np.repeat(np.eye(B_LOC, dtype=np.float32), B_LOC, axis=1)
    ident = np.eye(128, dtype=np.float32)

    nc = _get_nc()
    in_maps = []
    for c in range(N_CORES):
        b0 = c * B_LOC
        shard = np.ascontiguousarray(
            enc[:, b0:b0 + B_LOC, :].transpose(1, 0, 2)).reshape(ROWS, E2)
        # init[p, j] = c1[b] (+ masked: -1e10 * weight[t], so that after the
        # *1/weight scale the masked score lands at -1e10 -> exp underflows to 0)
        mseg = mk[b0:b0 + B_LOC].reshape(B_LOC, B_LOC, 128)     # [b, thi, p]
        mpen = np.where(mseg == 0, np.float32(NEG_INF), np.float32(0.0))
        wmat = weight.reshape(B_LOC, 128)                       # [thi, p]
        init_btp = c1[b0:b0 + B_LOC, None, None] + mpen * wmat[None, :, :]
        init = np.ascontiguousarray(
            init_btp.transpose(2, 0, 1).reshape(128, NT).astype(np.float32))
        init = (init * scl).astype(np.float32)   # fold the 1/weight scale in
        in_maps.append({
            "enc": shard, "w2rep": w2t_host, "init": init, "scl": scl,
            "ones": ones, "sel": sel, "ident": ident,
        })

    res = run_bass_kernel_spmd(nc, in_maps, list(range(N_CORES)))
    outs = [np.asarray(res.results[c]["out"]).reshape(B_LOC, T)
            for c in range(N_CORES)]
    attn = np.concatenate(outs, axis=0)            # [B, T]
    return attn[:, None, :].astype(np.float32)


# revision 22
# speedup vs baseline: 1.1062x; 1.1062x over previous
"""Trainium2 Bass kernel for the sparse-attention scoring module.

Math: the reference computes
    s     = concat([h, enc]) @ W_attn.T + b_attn        # [B, T, A]
    score = s @ v                                        # [B, T]
    score = score / weight ; masked -> -1e10 ; softmax over T

Since the A dimension is immediately contracted with v, the big matmul
collapses exactly:  score = concat @ (W_attn.T @ v) + b_attn @ v.
With w = W_attn.T @ v split into w1 (decoder half) and w2 (encoder half):
    score[b, t] = enc[t, b, :] . w2  +  (av[b] . w1 + b.v)
The only large tensor is encoder_outputs (268 MB fp32), so the kernel is
DMA-bound: each of the 8 cores streams its 8-batch shard (33.5 MB) through
SBUF in 512 KB transfers (alternating between the sync and scalar HWDGE DMA
rings, which is what saturates HBM) and does a fused multiply+reduce (STT
with accum) on the vector engine, then a small softmax tail. Scalar prep
(W_attn.T @ v, distance weights, mask penalties) happens on the host and
ships as tiny constant tensors.

Per-core data layout: the shard is re-ordered host-side to b-major rows
[8*1024, 1024] (row i = b*1024 + t). Row-tile j maps partition p to row
i = j*128 + p, i.e. b = j//8, t = (j%8)*128 + p. Scores accumulate into a
[128, 64] tile whose transpose [64, 128] is exactly the [8, 1024] output
row-major, so the final PE transpose + scale writes the output directly.
The -1e10 mask value is folded into the additive init constant as
-1e10 * weight[t], which the 1/weight scale restores to -1e10; exp then
underflows those lanes to exactly 0.
"""

import numpy as np

N_CORES = 8
B, T, E2, D, A = 64, 1024, 1024, 1024, 1024
B_LOC = B // N_CORES          # 8 batch rows per core
ROWS = B_LOC * T              # 8192 rows per core
NT = ROWS // 128              # 64 row-tiles of 128 rows
CHUNK = 1                     # row-tiles per DMA (512 KB transfers)
NEG_INF = -1.0e10

_CACHE = {}


def _build_nc():
    import concourse.bass as bass
    import concourse.tile as tile
    from concourse import bacc, mybir
    from contextlib import ExitStack

    f32 = mybir.dt.float32
    nc = bacc.Bacc("TRN2", target_bir_lowering=False, debug=False,
                   num_devices=N_CORES)

    enc = nc.dram_tensor("enc", [ROWS, E2], f32, kind="ExternalInput").ap()
    w2rep = nc.dram_tensor("w2rep", [128, E2], f32, kind="ExternalInput").ap()
    init = nc.dram_tensor("init", [128, NT], f32, kind="ExternalInput").ap()
    scl = nc.dram_tensor("scl", [128, NT], f32, kind="ExternalInput").ap()
    ones = nc.dram_tensor("ones", [128, 1], f32, kind="ExternalInput").ap()
    sel = nc.dram_tensor("sel", [B_LOC, NT], f32, kind="ExternalInput").ap()
    ident = nc.dram_tensor("ident", [128, 128], f32, kind="ExternalInput").ap()
    out = nc.dram_tensor("out", [NT, 128], f32, kind="ExternalOutput").ap()

    with tile.TileContext(nc) as tc, ExitStack() as ctx:
        const = ctx.enter_context(tc.tile_pool(name="const", bufs=1))
        encp = ctx.enter_context(tc.tile_pool(name="encp", bufs=6))
        prodp = ctx.enter_context(tc.tile_pool(name="prodp", bufs=2))
        small = ctx.enter_context(tc.tile_pool(name="small", bufs=1))
        psump = ctx.enter_context(tc.tile_pool(name="psump", bufs=1, space="PSUM"))

        # w2 replicated across partitions; issued on the sync HWDGE ring
        # while the first enc chunk streams on the scalar ring (the two
        # rings transfer concurrently). Remaining constants ride the SWDGE
        # (gpsimd) ring, off the enc stream.
        w2t = const.tile([128, E2], f32)
        nc.sync.dma_start(w2t[:], w2rep)
        sc = const.tile([128, NT], f32)
        nc.gpsimd.dma_start(sc[:], scl)
        ic = const.tile([128, NT], f32)
        nc.gpsimd.dma_start(ic[:], init)
        on = const.tile([128, 1], f32)
        nc.gpsimd.dma_start(on[:], ones)
        se = const.tile([B_LOC, NT], f32)
        nc.gpsimd.dma_start(se[:], sel)
        idt = const.tile([128, 128], f32)
        nc.gpsimd.dma_start(idt[:], ident)

        # Paired-row DMA: each transfer gives every partition TWO adjacent
        # DRAM rows (8 KB contiguous per partition instead of 4 KB), halving
        # descriptor count per byte. Partition p of pair jp holds rows
        # jp*256 + 2p and jp*256 + 2p + 1, so score column j = 2*jp + h maps
        # (p, j) -> row i = (j//2)*256 + 2p + (j%2). The per-batch column
        # grouping b = j//8 is preserved; the within-column t permutation is
        # compensated in the host-built sc/ic constants and undone on the
        # host when assembling the output.
        scores = small.tile([128, NT], f32)
        for jp in range(NT // 2):
            et = encp.tile([128, 2 * E2], f32, tag="enct")
            src = bass.AP(enc.tensor, jp * 256 * E2,
                          [[2 * E2, 128], [1, 2 * E2]])
            eng = nc.scalar if jp % 2 == 0 else nc.sync
            eng.dma_start(et[:], src)
            for h in range(2):
                j = jp * 2 + h
                pr = prodp.tile([128, E2], f32)
                # pr = (et_h * winv_col) * w2 ;  scores[:, j] = sum_e pr
                # (winv[t] is constant per partition within a score column,
                #  so the /weight scale rides the STT's per-partition scalar)
                nc.vector.scalar_tensor_tensor(
                    out=pr[:], in0=et[:, h * E2:(h + 1) * E2],
                    scalar=sc[:, j:j + 1], in1=w2t[:],
                    op0=mybir.AluOpType.mult, op1=mybir.AluOpType.mult,
                    accum_out=scores[:, j:j + 1],
                )

        # softmax tail: score' = scores + init*winv (host-folded); e = exp
        s3 = small.tile([128, NT], f32)
        nc.vector.tensor_add(s3[:], scores[:], ic[:])
        ex = small.tile([128, NT], f32)
        nc.scalar.activation(ex[:], s3[:], mybir.ActivationFunctionType.Exp)
        part = small.tile([128, B_LOC], f32)
        # one 3D-AP reduce: [128, (b thi)] -> sum over thi -> [128, b]
        nc.vector.reduce_sum(part[:], ex[:].rearrange("p (b t) -> p b t", b=B_LOC),
                             axis=mybir.AxisListType.X)
        ptot = psump.tile([B_LOC, 1], f32)
        nc.tensor.matmul(ptot[:], part[:], on[:], start=True, stop=True)
        rtot = small.tile([B_LOC, 1], f32)
        nc.vector.reciprocal(rtot[:], ptot[:])
        p64 = psump.tile([NT, 1], f32)
        nc.tensor.matmul(p64[:], se[:], rtot[:], start=True, stop=True)
        r64 = small.tile([NT, 1], f32)
        nc.scalar.copy(r64[:], p64[:])
        peT = psump.tile([NT, 128], f32)
        nc.tensor.transpose(peT[:], ex[:], idt[:])
        attn = small.tile([NT, 128], f32)
        nc.vector.tensor_scalar_mul(attn[:], peT[:], r64[:])
        nc.sync.dma_start(out, attn[:])

    nc.compile()
    return nc


def _get_nc():
    if "nc" not in _CACHE:
        _CACHE["nc"] = _build_nc()
    return _CACHE["nc"]


def _distance_weight(time_step: int, max_len: int) -> np.ndarray:
    left = np.arange(time_step, 0, -1) + 2
    right = np.arange(max_len - time_step) + 2
    return np.log2(np.concatenate([left, right]).astype(np.float32))


def kernel(attention_vector, encoder_outputs, W_attn, b_attn, v, mask,
           time_step, max_len) -> np.ndarray:
    from concourse.bass_utils import run_bass_kernel_spmd

    av = np.ascontiguousarray(np.asarray(attention_vector, dtype=np.float32))
    enc = np.asarray(encoder_outputs, dtype=np.float32)
    W = np.asarray(W_attn, dtype=np.float32)
    bb = np.asarray(b_attn, dtype=np.float32)
    vv = np.asarray(v, dtype=np.float32)
    mk = np.asarray(mask)
    ts = int(time_step)
    ml = int(max_len)
    assert av.shape == (B, D) and enc.shape == (T, B, E2)
    assert W.shape == (A, D + E2) and mk.shape == (B, T) and ml == T

    # Host-side scalar prep (tiny): collapse W/v/b, distance weights, mask.
    w = W.T @ vv                                   # [D+E2]
    w1, w2 = w[:D], np.ascontiguousarray(w[D:])
    w2t_host = np.ascontiguousarray(np.broadcast_to(w2, (128, E2)))
    bv = np.float32(bb @ vv)
    c1 = (av @ w1 + bv).astype(np.float32)         # [B]
    weight = _distance_weight(ts, ml)              # [T]
    winv = (np.float32(1.0) / weight).astype(np.float32)

    # Paired-row (p, j) -> (b_local, t) map: t = ((j//2)%4)*256 + 2p + j%2
    pgrid = np.arange(128)[:, None]                # [128, 1]
    jgrid = np.arange(NT)[None, :]                 # [1, NT]
    tmap = ((jgrid // 2) % 4) * 256 + 2 * pgrid + (jgrid % 2)   # [128, NT]
    bmap = jgrid // 8                              # [1, NT] local batch index
    scl = np.ascontiguousarray(winv[tmap])         # [128, NT]
    ones = np.ones((128, 1), dtype=np.float32)
    sel = np.repeat(np.eye(B_LOC, dtype=np.float32), B_LOC, axis=1)
    ident = np.eye(128, dtype=np.float32)

    nc = _get_nc()
    in_maps = []
    for c in range(N_CORES):
        b0 = c * B_LOC
        shard = np.ascontiguousarray(
            enc[:, b0:b0 + B_LOC, :].transpose(1, 0, 2)).reshape(ROWS, E2)
        # init[p, j] = (c1[b] + masked: -1e10 * weight[t]) / weight[t], so the
        # masked score lands at -1e10 -> exp underflows to exactly 0.
        mpen = np.where(mk[b0:b0 + B_LOC] == 0,
                        np.float32(NEG_INF), np.float32(0.0))   # [8, 1024]
        init_bt = c1[b0:b0 + B_LOC, None] + mpen * weight[None, :]  # [8, 1024]
        init = np.ascontiguousarray(
            (init_bt[bmap, tmap] * scl).astype(np.float32))     # [128, NT]
        in_maps.append({
            "enc": shard, "w2rep": w2t_host, "init": init, "scl": scl,
            "ones": ones, "sel": sel, "ident": ident,
        })

    res = run_bass_kernel_spmd(nc, in_maps, list(range(N_CORES)))
    # raw[j, p] = attn[b_local = j//8, t = ((j//2)%4)*256 + 2p + j%2]
    bo = bmap[0]                                   # [NT]
    to = tmap.T                                    # [NT, 128]
    outs = []
    for c in range(N_CORES):
        raw = np.asarray(res.results[c]["out"])    # [NT, 128]
        attn_c = np.empty((B_LOC, T), dtype=np.float32)
        attn_c[bo[:, None], to] = raw
        outs.append(attn_c)
    attn = np.concatenate(outs, axis=0)            # [B, T]
    return attn[:, None, :].astype(np.float32)


# revision 23
# speedup vs baseline: 1.1662x; 1.0542x over previous
"""Trainium2 Bass kernel for the sparse-attention scoring module.

Math: the reference computes
    s     = concat([h, enc]) @ W_attn.T + b_attn        # [B, T, A]
    score = s @ v                                        # [B, T]
    score = score / weight ; masked -> -1e10 ; softmax over T

Since the A dimension is immediately contracted with v, the big matmul
collapses exactly:  score = concat @ (W_attn.T @ v) + b_attn @ v.
With w = W_attn.T @ v split into w1 (decoder half) and w2 (encoder half):
    score[b, t] = enc[t, b, :] . w2  +  (av[b] . w1 + b.v)
The only large tensor is encoder_outputs (268 MB fp32), so the kernel is
DMA-bound: each of the 8 cores streams its 8-batch shard (33.5 MB) through
SBUF in 512 KB transfers (alternating between the sync and scalar HWDGE DMA
rings, which is what saturates HBM) and does a fused multiply+reduce (STT
with accum) on the vector engine, then a small softmax tail. Scalar prep
(W_attn.T @ v, distance weights, mask penalties) happens on the host and
ships as tiny constant tensors.

Per-core data layout: the shard is re-ordered host-side to b-major rows
[8*1024, 1024] (row i = b*1024 + t). Row-tile j maps partition p to row
i = j*128 + p, i.e. b = j//8, t = (j%8)*128 + p. Scores accumulate into a
[128, 64] tile whose transpose [64, 128] is exactly the [8, 1024] output
row-major, so the final PE transpose + scale writes the output directly.
The -1e10 mask value is folded into the additive init constant as
-1e10 * weight[t], which the 1/weight scale restores to -1e10; exp then
underflows those lanes to exactly 0.
"""

import numpy as np

N_CORES = 8
B, T, E2, D, A = 64, 1024, 1024, 1024, 1024
B_LOC = B // N_CORES          # 8 batch rows per core
ROWS = B_LOC * T              # 8192 rows per core
NT = ROWS // 128              # 64 row-tiles of 128 rows
CHUNK = 1                     # row-tiles per DMA (512 KB transfers)
NEG_INF = -1.0e10

_CACHE = {}


def _build_nc():
    import concourse.bass as bass
    import concourse.tile as tile
    from concourse import bacc, mybir
    from contextlib import ExitStack

    f32 = mybir.dt.float32
    nc = bacc.Bacc("TRN2", target_bir_lowering=False, debug=False,
                   num_devices=N_CORES)

    enc = nc.dram_tensor("enc", [ROWS, E2], f32, kind="ExternalInput").ap()
    w2rep = nc.dram_tensor("w2rep", [128, E2], f32, kind="ExternalInput").ap()
    init = nc.dram_tensor("init", [128, NT], f32, kind="ExternalInput").ap()
    scl = nc.dram_tensor("scl", [128, NT], f32, kind="ExternalInput").ap()
    ones = nc.dram_tensor("ones", [128, 1], f32, kind="ExternalInput").ap()
    sel = nc.dram_tensor("sel", [B_LOC, NT], f32, kind="ExternalInput").ap()
    ident = nc.dram_tensor("ident", [128, 128], f32, kind="ExternalInput").ap()
    out = nc.dram_tensor("out", [NT, 128], f32, kind="ExternalOutput").ap()

    with tile.TileContext(nc) as tc, ExitStack() as ctx:
        const = ctx.enter_context(tc.tile_pool(name="const", bufs=1))
        encp = ctx.enter_context(tc.tile_pool(name="encp", bufs=6))
        prodp = ctx.enter_context(tc.tile_pool(name="prodp", bufs=2))
        small = ctx.enter_context(tc.tile_pool(name="small", bufs=1))
        psump = ctx.enter_context(tc.tile_pool(name="psump", bufs=1, space="PSUM"))

        # w2 replicated across partitions; issued on the sync HWDGE ring
        # while the first enc chunk streams on the scalar ring (the two
        # rings transfer concurrently). Remaining constants ride the SWDGE
        # (gpsimd) ring, off the enc stream.
        w2t = const.tile([128, E2], f32)
        nc.sync.dma_start(w2t[:], w2rep)
        sc = const.tile([128, NT], f32)
        nc.gpsimd.dma_start(sc[:], scl)
        ic = const.tile([128, NT], f32)
        nc.gpsimd.dma_start(ic[:], init)
        on = const.tile([128, 1], f32)
        nc.gpsimd.dma_start(on[:], ones)
        se = const.tile([B_LOC, NT], f32)
        nc.gpsimd.dma_start(se[:], sel)
        idt = const.tile([128, 128], f32)
        nc.gpsimd.dma_start(idt[:], ident)

        # Paired-row DMA: each transfer gives every partition TWO adjacent
        # DRAM rows (8 KB contiguous per partition instead of 4 KB), halving
        # descriptor count per byte. Partition p of pair jp holds rows
        # jp*256 + 2p and jp*256 + 2p + 1, so score column j = 2*jp + h maps
        # (p, j) -> row i = (j//2)*256 + 2p + (j%2). The per-batch column
        # grouping b = j//8 is preserved; the within-column t permutation is
        # compensated in the host-built sc/ic constants and undone on the
        # host when assembling the output.
        scores = small.tile([128, NT], f32)
        # chunk list: (first_col, n_cols, dram_row0, rows_per_partition)
        chunk_list = [(0, 1, 0, 1), (1, 1, 128, 1)]
        chunk_list += [(2 + 2 * k, 2, 256 + 256 * k, 2) for k in range(31)]
        for ci, (j0c, ncols, row0, rpp) in enumerate(chunk_list):
            et = encp.tile([128, 2 * E2], f32, tag="enct")
            src = bass.AP(enc.tensor, row0 * E2,
                          [[rpp * E2, 128], [1, rpp * E2]])
            eng = nc.scalar if ci % 2 == 0 else nc.sync
            eng.dma_start(et[:, :rpp * E2], src)
            for h in range(ncols):
                j = j0c + h
                pr = prodp.tile([128, E2], f32)
                # pr = (et_h * winv_col) * w2 ;  scores[:, j] = sum_e pr
                # (winv[t] is constant per partition within a score column,
                #  so the /weight scale rides the STT's per-partition scalar)
                nc.vector.scalar_tensor_tensor(
                    out=pr[:], in0=et[:, h * E2:(h + 1) * E2],
                    scalar=sc[:, j:j + 1], in1=w2t[:],
                    op0=mybir.AluOpType.mult, op1=mybir.AluOpType.mult,
                    accum_out=scores[:, j:j + 1],
                )

        # softmax tail: score' = scores + init*winv (host-folded); e = exp
        s3 = small.tile([128, NT], f32)
        nc.vector.tensor_add(s3[:], scores[:], ic[:])
        ex = small.tile([128, NT], f32)
        nc.scalar.activation(ex[:], s3[:], mybir.ActivationFunctionType.Exp)
        part = small.tile([128, B_LOC], f32)
        # one 3D-AP reduce: [128, (b thi)] -> sum over thi -> [128, b]
        nc.vector.reduce_sum(part[:], ex[:].rearrange("p (b t) -> p b t", b=B_LOC),
                             axis=mybir.AxisListType.X)
        ptot = psump.tile([B_LOC, 1], f32)
        nc.tensor.matmul(ptot[:], part[:], on[:], start=True, stop=True)
        rtot = small.tile([B_LOC, 1], f32)
        nc.vector.reciprocal(rtot[:], ptot[:])
        p64 = psump.tile([NT, 1], f32)
        nc.tensor.matmul(p64[:], se[:], rtot[:], start=True, stop=True)
        r64 = small.tile([NT, 1], f32)
        nc.scalar.copy(r64[:], p64[:])
        peT = psump.tile([NT, 128], f32)
        nc.tensor.transpose(peT[:], ex[:], idt[:])
        attn = small.tile([NT, 128], f32)
        nc.vector.tensor_scalar_mul(attn[:], peT[:], r64[:])
        nc.sync.dma_start(out, attn[:])

    nc.compile()
    return nc


def _get_nc():
    if "nc" not in _CACHE:
        _CACHE["nc"] = _build_nc()
    return _CACHE["nc"]


def _tmap():
    """(p, j) -> t map: cols 0-1 are natural single tiles (fast start);
    cols 2+ are paired rows starting at row 256."""
    p = np.arange(128)[:, None]
    j = np.arange(NT)[None, :]
    k = (j - 2) // 2
    t_pair = 256 * ((k + 1) % 4) + 2 * p + (j - 2) % 2
    t_single = j * 128 + p
    return np.where(j < 2, t_single, t_pair)


def _distance_weight(time_step: int, max_len: int) -> np.ndarray:
    left = np.arange(time_step, 0, -1) + 2
    right = np.arange(max_len - time_step) + 2
    return np.log2(np.concatenate([left, right]).astype(np.float32))


def kernel(attention_vector, encoder_outputs, W_attn, b_attn, v, mask,
           time_step, max_len) -> np.ndarray:
    from concourse.bass_utils import run_bass_kernel_spmd

    av = np.ascontiguousarray(np.asarray(attention_vector, dtype=np.float32))
    enc = np.asarray(encoder_outputs, dtype=np.float32)
    W = np.asarray(W_attn, dtype=np.float32)
    bb = np.asarray(b_attn, dtype=np.float32)
    vv = np.asarray(v, dtype=np.float32)
    mk = np.asarray(mask)
    ts = int(time_step)
    ml = int(max_len)
    assert av.shape == (B, D) and enc.shape == (T, B, E2)
    assert W.shape == (A, D + E2) and mk.shape == (B, T) and ml == T

    # Host-side scalar prep (tiny): collapse W/v/b, distance weights, mask.
    w = W.T @ vv                                   # [D+E2]
    w1, w2 = w[:D], np.ascontiguousarray(w[D:])
    w2t_host = np.ascontiguousarray(np.broadcast_to(w2, (128, E2)))
    bv = np.float32(bb @ vv)
    c1 = (av @ w1 + bv).astype(np.float32)         # [B]
    weight = _distance_weight(ts, ml)              # [T]
    winv = (np.float32(1.0) / weight).astype(np.float32)

    # (p, j) -> (b_local, t) map for the fast-start + paired-row layout
    tmap = _tmap()                                 # [128, NT]
    bmap = np.broadcast_to(np.arange(NT)[None, :] // 8, tmap.shape)  # local b
    scl = np.ascontiguousarray(winv[tmap].astype(np.float32))
    ones = np.ones((128, 1), dtype=np.float32)
    sel = np.repeat(np.eye(B_LOC, dtype=np.float32), B_LOC, axis=1)
    ident = np.eye(128, dtype=np.float32)

    nc = _get_nc()
    in_maps = []
    for c in range(N_CORES):
        b0 = c * B_LOC
        shard = np.ascontiguousarray(
            enc[:, b0:b0 + B_LOC, :].transpose(1, 0, 2)).reshape(ROWS, E2)
        # init[p, j] = (c1[b] + masked: -1e10 * weight[t]) / weight[t], so the
        # masked score lands at -1e10 -> exp underflows to exactly 0.
        mpen = np.where(mk[b0:b0 + B_LOC] == 0,
                        np.float32(NEG_INF), np.float32(0.0))   # [8, 1024]
        init_bt = c1[b0:b0 + B_LOC, None] + mpen * weight[None, :]  # [8, 1024]
        init = np.ascontiguousarray(
            (init_bt[bmap, tmap] * scl).astype(np.float32))     # [128, NT]
        in_maps.append({
            "enc": shard, "w2rep": w2t_host, "init": init, "scl": scl,
            "ones": ones, "sel": sel, "ident": ident,
        })

    res = run_bass_kernel_spmd(nc, in_maps, list(range(N_CORES)))
    # raw[j, p] = attn[b_local = j//8, t = ((j//2)%4)*256 + 2p + j%2]
    bo = bmap[0]                                   # [NT]
    to = tmap.T                                    # [NT, 128]
    outs = []
    for c in range(N_CORES):
        raw = np.asarray(res.results[c]["out"])    # [NT, 128]
        attn_c = np.empty((B_LOC, T), dtype=np.float32)
        attn_c[bo[:, None], to] = raw
        outs.append(attn_c)
    attn = np.concatenate(outs, axis=0)            # [B, T]
    return attn[:, None, :].astype(np.float32)
